# revision 4
# baseline (speedup 1.0000x reference)
"""Trainium2 Bass kernel for a single-layer GRU (PyTorch semantics), returning
the final hidden state h_T.

Problem: inputs (256, 2048, 50) fp32, W_ih/W_hh (150, 50), b_ih/b_hh (150,).
Strategy: data-parallel over 8 NeuronCores (32 sequences each). Per core the
recurrence runs in a transposed layout (hidden units on SBUF partitions, batch
on the free dimension). The input projection gx = W_ih @ x_t^T for 16 time
steps at a time is computed by one PE matmul into a PSUM bank (start=True) and
each step's recurrent matmul accumulates its gh contribution into the same
bank slice (start=False) — gx is never materialized to HBM. Biases are folded
via an appended ones-channel on x and a ones-row on the h state tile.
"""
import sys

sys.path.insert(0, "/opt/trn_rl_repo")
import numpy as np
from contextlib import ExitStack

import concourse.bass as bass
import concourse.bacc as bacc
import concourse.tile as tile
from concourse import mybir
from concourse.bass_utils import run_bass_kernel_spmd

F32 = mybir.dt.float32
AF = mybir.ActivationFunctionType

N_CORES = 8
B_FULL, T, H = 256, 2048, 50
B = B_FULL // N_CORES  # 32 sequences per core
Tc = 16  # time steps per PSUM chunk (16*32 = 512 fp32 = one PSUM bank)
C = T // Tc
K = H + 1  # hidden/input dim augmented with a ones row (bias folding)
N = Tc * B
# z gate lives at partition offset 64 inside the padded 128-partition r/z
# block (engine SBUF access must start at a multiple-of-32 partition).
ZOFF = 64


def _build_nc(repeats=1):
    """repeats > 1 reruns the whole computation sequentially inside one NEFF
    (used only by the test harness for wall-clock timing amplification)."""
    nc = bacc.Bacc("TRN2", target_bir_lowering=False, debug=False,
                   num_devices=N_CORES)
    xt = nc.dram_tensor("xt", (C, K, N), F32, kind="ExternalInput")
    wxrz = nc.dram_tensor("wxrz", (K, 128), F32, kind="ExternalInput")
    wxn = nc.dram_tensor("wxn", (K, H), F32, kind="ExternalInput")
    whrz = nc.dram_tensor("whrz", (K, 128), F32, kind="ExternalInput")
    whn = nc.dram_tensor("whn", (K, H), F32, kind="ExternalInput")
    h0init = nc.dram_tensor("h0init", (K, B), F32, kind="ExternalInput")
    y = nc.dram_tensor("y", (H, B), F32, kind="ExternalOutput")

    with ExitStack() as ctx:
        tc_ctx = ctx.enter_context(tile.TileContext(nc))
        consts = ctx.enter_context(tc_ctx.tile_pool(name="consts", bufs=1))
        xpool = ctx.enter_context(tc_ctx.tile_pool(name="xp", bufs=3))
        prz_pool = ctx.enter_context(
            tc_ctx.tile_pool(name="prz", bufs=2, space="PSUM"))
        pn_pool = ctx.enter_context(
            tc_ctx.tile_pool(name="pn", bufs=2, space="PSUM"))
        pnh_pool = ctx.enter_context(
            tc_ctx.tile_pool(name="pnh", bufs=2, space="PSUM"))
        gates = ctx.enter_context(tc_ctx.tile_pool(name="gates", bufs=3))
        tmps = ctx.enter_context(tc_ctx.tile_pool(name="tmps", bufs=3))

        wxrz_sb = consts.tile([K, 128], F32, tag="wxrz")
        wxn_sb = consts.tile([K, H], F32, tag="wxn")
        whrz_sb = consts.tile([K, 128], F32, tag="whrz")
        whn_sb = consts.tile([K, H], F32, tag="whn")
        nc.sync.dma_start(out=wxrz_sb[:], in_=wxrz[:, :])
        nc.sync.dma_start(out=wxn_sb[:], in_=wxn[:, :])
        nc.sync.dma_start(out=whrz_sb[:], in_=whrz[:, :])
        nc.sync.dma_start(out=whn_sb[:], in_=whn[:, :])

        h_bufs = [
            consts.tile([K, B], F32, tag=f"h{i}", name=f"h{i}") for i in range(2)
        ]
        for _rep in range(repeats):
          for hb in h_bufs:
            nc.sync.dma_start(out=hb[:], in_=h0init[:, :])

          for c in range(C):
            xt_sb = xpool.tile([K, N], F32, tag="xt")
            nc.sync.dma_start(out=xt_sb[:], in_=xt[c, :, :])
            prz = prz_pool.tile([128, N], F32, tag="prz")
            pn = pn_pool.tile([H, N], F32, tag="pn")
            nc.tensor.matmul(prz[:], wxrz_sb[:], xt_sb[:], start=True,
                             stop=False, skip_group_check=True)
            nc.tensor.matmul(pn[:], wxn_sb[:], xt_sb[:], start=True,
                             stop=True, skip_group_check=True)
            for ti in range(Tc):
                t = c * Tc + ti
                hc = h_bufs[t % 2]
                hn = h_bufs[(t + 1) % 2]
                sl = bass.ts(ti, B)
                pnh = pnh_pool.tile([H, B], F32, tag="pnh")
                nc.tensor.matmul(prz[:, sl], whrz_sb[:], hc[:], start=False,
                                 stop=True, skip_group_check=True)
                nc.tensor.matmul(pnh[:], whn_sb[:], hc[:], start=True,
                                 stop=True, skip_group_check=True)
                rz = gates.tile([128, B], F32, tag="rz")
                nc.scalar.activation(rz[:], prz[:, sl], AF.Sigmoid)
                u = tmps.tile([H, B], F32, tag="u")
                nc.vector.tensor_mul(u[:], rz[0:H, :], pnh[:])  # r * ghn
                nc.vector.tensor_add(pnh[:], u[:], pn[:, sl])  # + gxn
                n_sb = tmps.tile([H, B], F32, tag="n")
                nc.scalar.activation(n_sb[:], pnh[:], AF.Tanh)
                d = tmps.tile([128, B], F32, tag="d")
                # h - n, written at partition offset ZOFF so the next
                # tensor_tensor sees equal SBUF base partitions
                nc.vector.tensor_sub(d[ZOFF : ZOFF + H, :], hc[0:H, :], n_sb[:])
                yv = tmps.tile([H, B], F32, tag="yv")
                nc.vector.tensor_mul(yv[:], rz[ZOFF : ZOFF + H, :],
                                     d[ZOFF : ZOFF + H, :])
                nc.vector.tensor_add(hn[0:H, :], n_sb[:], yv[:])
        h_final = h_bufs[T % 2]
        nc.sync.dma_start(out=y[:, :], in_=h_final[0:H, :])
    nc.compile()
    return nc


def _prep_in_maps(inputs, W_ih, W_hh, b_ih, b_hh):
    inputs = np.ascontiguousarray(inputs, dtype=np.float32)
    W_ih = np.asarray(W_ih, dtype=np.float32)
    W_hh = np.asarray(W_hh, dtype=np.float32)
    b_ih = np.asarray(b_ih, dtype=np.float32)
    b_hh = np.asarray(b_hh, dtype=np.float32)

    wxrz = np.zeros((K, 128), np.float32)
    wxrz[0:H, 0:H] = W_ih[0:H].T
    wxrz[0:H, ZOFF : ZOFF + H] = W_ih[H : 2 * H].T
    wxrz[H, 0:H] = b_ih[0:H] + b_hh[0:H]
    wxrz[H, ZOFF : ZOFF + H] = b_ih[H : 2 * H] + b_hh[H : 2 * H]
    wxn = np.empty((K, H), np.float32)
    wxn[0:H] = W_ih[2 * H : 3 * H].T
    wxn[H] = b_ih[2 * H : 3 * H]
    whrz = np.zeros((K, 128), np.float32)
    whrz[0:H, 0:H] = W_hh[0:H].T
    whrz[0:H, ZOFF : ZOFF + H] = W_hh[H : 2 * H].T
    whn = np.empty((K, H), np.float32)
    whn[0:H] = W_hh[2 * H : 3 * H].T
    whn[H] = b_hh[2 * H : 3 * H]
    h0init = np.zeros((K, B), np.float32)
    h0init[H] = 1.0

    in_maps = []
    for core in range(N_CORES):
        xc = inputs[core * B : (core + 1) * B]  # (B, T, H)
        xa = np.concatenate([xc, np.ones((B, T, 1), np.float32)], axis=2)
        xt = np.ascontiguousarray(
            xa.reshape(B, C, Tc, K).transpose(1, 3, 2, 0).reshape(C, K, N)
        )
        in_maps.append({"xt": xt, "wxrz": wxrz, "wxn": wxn, "whrz": whrz,
                        "whn": whn, "h0init": h0init})
    return in_maps


_NC_CACHE = []


def kernel(inputs, W_ih, W_hh, b_ih, b_hh, z=0, **_ignored):
    if np.asarray(inputs).ndim == 2:
        inputs = np.asarray(inputs)[None]
    if not _NC_CACHE:
        _NC_CACHE.append(_build_nc())
    nc = _NC_CACHE[0]
    in_maps = _prep_in_maps(inputs, W_ih, W_hh, b_ih, b_hh)
    res = run_bass_kernel_spmd(nc, in_maps, core_ids=list(range(N_CORES)))
    out = np.empty((B_FULL, H), np.float32)
    for core in range(N_CORES):
        out[core * B : (core + 1) * B] = res.results[core]["y"].T
    return out


if __name__ == "__main__":
    rng = np.random.default_rng(0)
    s = 1.0 / np.sqrt(H)
    demo = {
        "inputs": rng.standard_normal((B_FULL, T, H), dtype=np.float32),
        "W_ih": rng.uniform(-s, s, (3 * H, H)).astype(np.float32),
        "W_hh": rng.uniform(-s, s, (3 * H, H)).astype(np.float32),
        "b_ih": rng.uniform(-s, s, (3 * H,)).astype(np.float32),
        "b_hh": rng.uniform(-s, s, (3 * H,)).astype(np.float32),
        "z": 0,
    }
    out = kernel(**demo)
    print("kernel output", out.shape, out.dtype, out[0, :4])
